# revision 9
# baseline (speedup 1.0000x reference)
"""Multi-head attention Trainium2 kernel (B=4, S=2048, D=1024, H=16, HD=64).

Sharding: core c handles batch b = c // 2, head-group g = c % 2 (8 heads).
Each core computes its heads' Q/K/V projections, masked softmax attention
(bounded by max valid_len), and a row-parallel partial of the output
projection.  The host sums the two partials per batch.

Layout strategy (per core):
  - scores are computed TRANSPOSED: S.T[kpos, q] = K_h @ Q_h.T so that the
    valid_len mask is a per-partition bias fused into the ACT exp, and the
    attention*V matmul needs no transposes anywhere.
  - head pairs are row-packed on the PE (contraction dh=64 at array rows
    0-63 / 64-127 concurrently).
  - softmax denominators come from a ones-column appended to V (the extra
    PSUM row costs no matmul time).
  - no max-subtraction in softmax: scores are O(1) here, exp is safe.
"""

import os
import threading

import numpy as np
import ml_dtypes

B, S, D, H = 4, 2048, 1024, 16
HD = 64
P = 128
DG = 512          # head dims per core (8 heads)
NPAIR = 4         # head pairs per core
MASK_VALUE = -1e6
QT_TILES = S // 512   # 4 query tiles of 512
DC = D // P           # 8 contraction chunks for projections

_BF16 = ml_dtypes.bfloat16

_build_cache = {}


def _build_nc(nkc: int):
    """Build the Bass program, parameterized by number of 128-wide key chunks."""
    import concourse.bass as bass
    import concourse.bacc as bacc
    import concourse.tile as tile
    from concourse import mybir

    f32 = mybir.dt.float32
    f32r = mybir.dt.float32r
    bf16 = mybir.dt.bfloat16
    EXP = mybir.ActivationFunctionType.Exp

    vlp = nkc * P
    # K/V projection free-dim tiles over key positions
    kt_widths = []
    rem = vlp
    while rem > 0:
        w = min(512, rem)
        kt_widths.append(w)
        rem -= w

    nc = bacc.Bacc("TRN2", target_bir_lowering=False)

    xq = nc.dram_tensor("xq", [D, S], bf16, kind="ExternalInput")
    xk = nc.dram_tensor("xk", [D, vlp], bf16, kind="ExternalInput")
    xv = nc.dram_tensor("xv", [D, vlp], bf16, kind="ExternalInput")
    wq = nc.dram_tensor("wq", [D, DG], bf16, kind="ExternalInput")  # pre-scaled 1/8
    wk = nc.dram_tensor("wk", [D, DG], bf16, kind="ExternalInput")
    wv = nc.dram_tensor("wv", [D, DG], bf16, kind="ExternalInput")
    wo = nc.dram_tensor("wo", [DG, D], bf16, kind="ExternalInput")
    maskd = nc.dram_tensor("maskd", [P, nkc], f32, kind="ExternalInput")
    y = nc.dram_tensor("y", [S, D], f32, kind="ExternalOutput")

    xq_r = xq.rearrange("(dc p) s -> p dc s", p=P)
    xk_r = xk.rearrange("(dc p) s -> p dc s", p=P)
    xv_r = xv.rearrange("(dc p) s -> p dc s", p=P)
    wq_r = wq.rearrange("(dc p) m -> p dc m", p=P)
    wk_r = wk.rearrange("(dc p) m -> p dc m", p=P)
    wv_r = wv.rearrange("(dc p) m -> p dc m", p=P)
    wo_r = wo.rearrange("(pr p) o -> p pr o", p=P)
    y_r = y.rearrange("(sc p) o -> p sc o", p=P)

    with tile.TileContext(nc) as tc:
        with (
            tc.tile_pool(name="persist", bufs=1) as persist,
            tc.tile_pool(name="xstream", bufs=3) as xstream,
            tc.tile_pool(name="work", bufs=3) as work,
            tc.tile_pool(name="ps_big", bufs=2, space="PSUM") as ps_big,
            tc.tile_pool(name="ps_av", bufs=2, space="PSUM") as ps_av,
        ):
            # ---- resident tensors -------------------------------------
            wq_sb = persist.tile([P, DC, DG], bf16)
            wk_sb = persist.tile([P, DC, DG], bf16)
            wv_sb = persist.tile([P, DC, DG], bf16)
            wo_sb = persist.tile([P, NPAIR, D], bf16)
            mask_sb = persist.tile([P, nkc], f32)
            qt_sb = persist.tile([P, NPAIR, S], bf16)       # Q.T per pair
            kt_sb = persist.tile([P, NPAIR, vlp], bf16)     # K.T per pair
            v_sb = persist.tile([P, nkc, NPAIR, 130], bf16)  # [VA|1|VB|1] blocks
            ot_sb = persist.tile([P, NPAIR, S], bf16)       # normalized attn out
            ones_sb = persist.tile([P, 64], bf16)

            nc.sync.dma_start(wq_sb, wq_r)
            nc.sync.dma_start(wk_sb, wk_r)
            nc.sync.dma_start(wv_sb, wv_r)
            nc.sync.dma_start(wo_sb, wo_r)
            nc.sync.dma_start(mask_sb, maskd[:, :])
            nc.vector.memset(ones_sb, 1.0)
            # ones columns of the augmented V blocks
            nc.vector.memset(v_sb[:, :, :, 64:65], 1.0)
            nc.vector.memset(v_sb[:, :, :, 129:130], 1.0)

            # ---- V projection: V[s, dh] (s on partitions) -------------
            for sc in range(nkc):
                xv_t = xstream.tile([P, DC, P], bf16, tag="xv")
                nc.gpsimd.dma_start(xv_t, xv_r[:, :, sc * P:(sc + 1) * P])
                v_ps = ps_big.tile([P, DG], f32, tag="big")
                for dc in range(DC):
                    nc.tensor.matmul(
                        v_ps, lhsT=xv_t[:, dc, :], rhs=wv_sb[:, dc, :],
                        start=(dc == 0), stop=(dc == DC - 1),
                    )
                for pr in range(NPAIR):
                    nc.vector.tensor_copy(
                        out=v_sb[:, sc, pr, 0:64],
                        in_=v_ps[:, pr * P:pr * P + 64],
                    )
                    nc.vector.tensor_copy(
                        out=v_sb[:, sc, pr, 65:129],
                        in_=v_ps[:, pr * P + 64:(pr + 1) * P],
                    )

            # ---- Q.T projection: QT[dh, s] (dh on partitions) ---------
            for st in range(QT_TILES):
                xq_t = xstream.tile([P, DC, 512], bf16, tag="xq")
                nc.gpsimd.dma_start(xq_t, xq_r[:, :, st * 512:(st + 1) * 512])
                for pr in range(NPAIR):
                    q_ps = ps_big.tile([P, 512], f32, tag="big")
                    for dc in range(DC):
                        nc.tensor.matmul(
                            q_ps, lhsT=wq_sb[:, dc, pr * P:(pr + 1) * P],
                            rhs=xq_t[:, dc, :],
                            start=(dc == 0), stop=(dc == DC - 1),
                        )
                    nc.vector.tensor_copy(
                        out=qt_sb[:, pr, st * 512:(st + 1) * 512], in_=q_ps
                    )

            # ---- K.T projection ---------------------------------------
            for kt, w in enumerate(kt_widths):
                xk_t = xstream.tile([P, DC, 512], bf16, tag="xq")
                nc.gpsimd.dma_start(
                    xk_t[:, :, :w], xk_r[:, :, kt * 512:kt * 512 + w]
                )
                for pr in range(NPAIR):
                    k_ps = ps_big.tile([P, 512], f32, tag="big")
                    for dc in range(DC):
                        nc.tensor.matmul(
                            k_ps[:, :w], lhsT=wk_sb[:, dc, pr * P:(pr + 1) * P],
                            rhs=xk_t[:, dc, :w],
                            start=(dc == 0), stop=(dc == DC - 1),
                        )
                    nc.vector.tensor_copy(
                        out=kt_sb[:, pr, kt * 512:kt * 512 + w], in_=k_ps[:, :w]
                    )

            # ---- attention per head pair ------------------------------
            for pr in range(NPAIR):
                for qt in range(QT_TILES):
                    qsl = slice(qt * 512, (qt + 1) * 512)
                    av_ps = ps_av.tile([65, 1024], f32, tag="av")
                    for kc in range(nkc):
                        ksl = slice(kc * P, (kc + 1) * P)
                        sc_ps = ps_big.tile([P, 1024], f32, tag="big")
                        # scores.T for head A (rows 0-63) and B (rows 64-127)
                        nc.tensor.matmul(
                            sc_ps[:, 0:512],
                            lhsT=kt_sb[0:64, pr, ksl], rhs=qt_sb[0:64, pr, qsl],
                        )
                        nc.tensor.matmul(
                            sc_ps[:, 512:1024],
                            lhsT=kt_sb[64:128, pr, ksl], rhs=qt_sb[64:128, pr, qsl],
                        )
                        exps = work.tile([P, 1024], bf16, tag="exps")
                        nc.scalar.activation(
                            out=exps, in_=sc_ps, func=EXP,
                            bias=mask_sb[:, kc:kc + 1], scale=1.0,
                        )
                        nc.tensor.matmul(
                            av_ps[0:65, 0:512],
                            lhsT=v_sb[:, kc, pr, 0:65], rhs=exps[:, 0:512],
                            start=(kc == 0), stop=(kc == nkc - 1),
                        )
                        nc.tensor.matmul(
                            av_ps[0:65, 512:1024],
                            lhsT=v_sb[:, kc, pr, 65:130], rhs=exps[:, 512:1024],
                            start=(kc == 0), stop=(kc == nkc - 1),
                        )
                    # normalize: rows 64 hold the softmax denominators.
                    # broadcast denom across 64 partitions via PE ones-matmul,
                    # then reciprocal on the broadcast tile.
                    dn_bf = work.tile([65, 1024], bf16, tag="dnbf")
                    nc.vector.tensor_copy(
                        out=dn_bf[64:65, :], in_=av_ps[64:65, :]
                    )
                    bc_ps = ps_big.tile([P, 1024], f32, tag="big")
                    nc.tensor.matmul(
                        bc_ps[0:64, 0:512],
                        lhsT=ones_sb[64:65, 0:64],
                        rhs=dn_bf[64:65, 0:512],
                    )
                    nc.tensor.matmul(
                        bc_ps[0:64, 512:1024],
                        lhsT=ones_sb[64:65, 0:64],
                        rhs=dn_bf[64:65, 512:1024],
                    )
                    bc_sb = work.tile([64, 1024], f32, tag="bc")
                    nc.vector.reciprocal(out=bc_sb, in_=bc_ps[0:64, :])
                    nc.vector.tensor_mul(
                        out=ot_sb[0:64, pr, qsl],
                        in0=av_ps[0:64, 0:512], in1=bc_sb[:, 0:512],
                    )
                    otB = work.tile([64, 512], bf16, tag="otB")
                    nc.vector.tensor_mul(
                        out=otB,
                        in0=av_ps[0:64, 512:1024], in1=bc_sb[:, 512:1024],
                    )
                    nc.sync.dma_start(out=ot_sb[64:128, pr, qsl], in_=otB)

            # ---- output projection (row-parallel partial) -------------
            for sch in range(S // P):
                y_ps = ps_big.tile([P, D], f32, tag="big")
                for pr in range(NPAIR):
                    for nh in range(2):
                        nc.tensor.matmul(
                            y_ps[:, nh * 512:(nh + 1) * 512],
                            lhsT=ot_sb[:, pr, sch * P:(sch + 1) * P],
                            rhs=wo_sb[:, pr, nh * 512:(nh + 1) * 512],
                            start=(pr == 0), stop=(pr == NPAIR - 1),
                        )
                y_sb = work.tile([P, D], f32, tag="ysb")
                nc.vector.tensor_copy(out=y_sb, in_=y_ps)
                nc.sync.dma_start(out=y_r[:, sch, :], in_=y_sb)

    nc.finalize()
    return nc


def _prep_core_inputs(inputs, nkc):
    """Host-side shard prep: per-core input dict for core c = (batch, group)."""
    vlp = nkc * P
    q = np.asarray(inputs["queries"], np.float32)
    k = np.asarray(inputs["keys"], np.float32)
    v = np.asarray(inputs["values"], np.float32)
    vl = np.asarray(inputs["valid_lens"]).astype(np.int64)
    Wq = np.asarray(inputs["Wq"], np.float32)
    Wk = np.asarray(inputs["Wk"], np.float32)
    Wv = np.asarray(inputs["Wv"], np.float32)
    Wo = np.asarray(inputs["Wo"], np.float32)

    in_maps = []
    for c in range(8):
        b, g = c // 2, c % 2
        rows = slice(g * DG, (g + 1) * DG)
        kpos = np.arange(P)[:, None] + P * np.arange(nkc)[None, :]
        mask = np.where(kpos < vl[b], 0.0, MASK_VALUE).astype(np.float32)
        in_maps.append({
            "xq": np.ascontiguousarray(q[b].T).astype(_BF16),
            "xk": np.ascontiguousarray(k[b, :vlp].T).astype(_BF16),
            "xv": np.ascontiguousarray(v[b, :vlp].T).astype(_BF16),
            "wq": np.ascontiguousarray(Wq[rows].T / 8.0).astype(_BF16),
            "wk": np.ascontiguousarray(Wk[rows].T).astype(_BF16),
            "wv": np.ascontiguousarray(Wv[rows].T).astype(_BF16),
            "wo": np.ascontiguousarray(Wo[:, rows].T).astype(_BF16),
            "maskd": mask,
        })
    return in_maps


def kernel(**inputs):
    from concourse.bass_utils import run_bass_kernel_spmd

    vl = np.asarray(inputs["valid_lens"]).astype(np.int64)
    nkc = int(min(S, max(1, int(vl.max()))) + P - 1) // P

    if nkc not in _build_cache:
        _build_cache[nkc] = _build_nc(nkc)
    nc = _build_cache[nkc]

    in_maps = _prep_core_inputs(inputs, nkc)
    res = run_bass_kernel_spmd(
        nc, in_maps, core_ids=list(range(8)),
        trace=bool(int(os.environ.get("MHA_TRACE", "0"))),
    )
    out = np.empty((B, S, D), np.float32)
    for b in range(B):
        out[b] = res.results[2 * b]["y"] + res.results[2 * b + 1]["y"]
    kernel.last_results = res
    return out


if __name__ == "__main__":
    rng = np.random.default_rng(0)
    ins = {
        "queries": rng.standard_normal((B, S, D), np.float32),
        "keys": rng.standard_normal((B, S, D), np.float32),
        "values": rng.standard_normal((B, S, D), np.float32),
        "valid_lens": np.array([288, 576, 1749, 255], np.int32),
        "Wq": rng.uniform(-1 / 32, 1 / 32, (D, D)).astype(np.float32),
        "Wk": rng.uniform(-1 / 32, 1 / 32, (D, D)).astype(np.float32),
        "Wv": rng.uniform(-1 / 32, 1 / 32, (D, D)).astype(np.float32),
        "Wo": rng.uniform(-1 / 32, 1 / 32, (D, D)).astype(np.float32),
    }
    out = kernel(**ins)
    print("kernel ran, out", out.shape, out.dtype, float(np.abs(out).mean()))


# revision 10
# speedup vs baseline: 1.1744x; 1.1744x over previous
"""Multi-head attention Trainium2 kernel (B=4, S=2048, D=1024, H=16, HD=64).

Sharding: core c handles batch b = c // 2, head-group g = c % 2 (8 heads).
Each core computes its heads' Q/K/V projections, masked softmax attention
(bounded by max valid_len), and a row-parallel partial of the output
projection.  The host sums the two partials per batch.

Layout strategy (per core):
  - scores are computed TRANSPOSED: S.T[kpos, q] = K_h @ Q_h.T so that the
    valid_len mask is a per-partition bias fused into the ACT exp, and the
    attention*V matmul needs no transposes anywhere.
  - head pairs are row-packed on the PE (contraction dh=64 at array rows
    0-63 / 64-127 concurrently).
  - softmax denominators come from a ones-column appended to V (the extra
    PSUM row costs no matmul time).
  - no max-subtraction in softmax: scores are O(1) here, exp is safe.
"""

import os
import threading

import numpy as np
import ml_dtypes

B, S, D, H = 4, 2048, 1024, 16
HD = 64
P = 128
DG = 512          # head dims per core (8 heads)
NPAIR = 4         # head pairs per core
MASK_VALUE = -1e6
QT_TILES = S // 512   # 4 query tiles of 512
DC = D // P           # 8 contraction chunks for projections

_BF16 = ml_dtypes.bfloat16

_build_cache = {}


def _build_nc(nkc: int):
    """Build the Bass program, parameterized by number of 128-wide key chunks."""
    import concourse.bass as bass
    import concourse.bacc as bacc
    import concourse.tile as tile
    from concourse import mybir

    f32 = mybir.dt.float32
    f32r = mybir.dt.float32r
    bf16 = mybir.dt.bfloat16
    EXP = mybir.ActivationFunctionType.Exp

    vlp = nkc * P
    # K/V projection free-dim tiles over key positions
    kt_widths = []
    rem = vlp
    while rem > 0:
        w = min(512, rem)
        kt_widths.append(w)
        rem -= w

    nc = bacc.Bacc("TRN2", target_bir_lowering=False)

    xq = nc.dram_tensor("xq", [D, S], bf16, kind="ExternalInput")
    xk = nc.dram_tensor("xk", [D, vlp], bf16, kind="ExternalInput")
    xv = nc.dram_tensor("xv", [D, vlp], bf16, kind="ExternalInput")
    wq = nc.dram_tensor("wq", [D, DG], bf16, kind="ExternalInput")  # pre-scaled 1/8
    wk = nc.dram_tensor("wk", [D, DG], bf16, kind="ExternalInput")
    wv = nc.dram_tensor("wv", [D, DG], bf16, kind="ExternalInput")
    wo = nc.dram_tensor("wo", [DG, D], bf16, kind="ExternalInput")
    maskd = nc.dram_tensor("maskd", [P, nkc], f32, kind="ExternalInput")
    y = nc.dram_tensor("y", [S, D], f32, kind="ExternalOutput")

    xq_r = xq.rearrange("(dc p) s -> p dc s", p=P)
    xk_r = xk.rearrange("(dc p) s -> p dc s", p=P)
    xv_r = xv.rearrange("(dc p) s -> p dc s", p=P)
    wq_r = wq.rearrange("(dc p) m -> p dc m", p=P)
    wk_r = wk.rearrange("(dc p) m -> p dc m", p=P)
    wv_r = wv.rearrange("(dc p) m -> p dc m", p=P)
    wo_r = wo.rearrange("(pr p) o -> p pr o", p=P)
    y_r = y.rearrange("(sc p) o -> p sc o", p=P)

    with tile.TileContext(nc) as tc:
        with (
            tc.tile_pool(name="persist", bufs=1) as persist,
            tc.tile_pool(name="xstream", bufs=3) as xstream,
            tc.tile_pool(name="work", bufs=3) as work,
            tc.tile_pool(name="ps_big", bufs=2, space="PSUM") as ps_big,
            tc.tile_pool(name="ps_av", bufs=2, space="PSUM") as ps_av,
        ):
            # ---- resident tensors -------------------------------------
            wq_sb = persist.tile([P, DC, DG], bf16)
            wk_sb = persist.tile([P, DC, DG], bf16)
            wv_sb = persist.tile([P, DC, DG], bf16)
            wo_sb = persist.tile([P, NPAIR, D], bf16)
            mask_sb = persist.tile([P, nkc], f32)
            qt_sb = persist.tile([P, NPAIR, S], bf16)       # Q.T per pair
            kt_sb = persist.tile([P, NPAIR, vlp], bf16)     # K.T per pair
            v_sb = persist.tile([P, nkc, NPAIR, 130], bf16)  # [VA|1|VB|1] blocks
            ot_sb = persist.tile([P, NPAIR, S], bf16)       # normalized attn out
            ones_sb = persist.tile([P, 64], bf16)

            nc.sync.dma_start(wq_sb, wq_r)
            nc.sync.dma_start(wk_sb, wk_r)
            nc.sync.dma_start(wv_sb, wv_r)
            nc.sync.dma_start(wo_sb, wo_r)
            nc.sync.dma_start(mask_sb, maskd[:, :])
            nc.vector.memset(ones_sb, 1.0)
            # ones columns of the augmented V blocks
            nc.vector.memset(v_sb[:, :, :, 64:65], 1.0)
            nc.vector.memset(v_sb[:, :, :, 129:130], 1.0)

            # ---- V projection: V[s, dh] (s on partitions) -------------
            for sc in range(nkc):
                xv_t = xstream.tile([P, DC, P], bf16, tag="xv")
                nc.gpsimd.dma_start(xv_t, xv_r[:, :, sc * P:(sc + 1) * P])
                v_ps = ps_big.tile([P, DG], f32, tag="big")
                for dc in range(DC):
                    nc.tensor.matmul(
                        v_ps, lhsT=xv_t[:, dc, :], rhs=wv_sb[:, dc, :],
                        start=(dc == 0), stop=(dc == DC - 1),
                    )
                for pr in range(NPAIR):
                    nc.vector.tensor_copy(
                        out=v_sb[:, sc, pr, 0:64],
                        in_=v_ps[:, pr * P:pr * P + 64],
                    )
                    nc.vector.tensor_copy(
                        out=v_sb[:, sc, pr, 65:129],
                        in_=v_ps[:, pr * P + 64:(pr + 1) * P],
                    )

            # ---- Q.T projection: QT[dh, s] (dh on partitions) ---------
            for st in range(QT_TILES):
                xq_t = xstream.tile([P, DC, 512], bf16, tag="xq")
                nc.gpsimd.dma_start(xq_t, xq_r[:, :, st * 512:(st + 1) * 512])
                for pr in range(NPAIR):
                    q_ps = ps_big.tile([P, 512], f32, tag="big")
                    for dc in range(DC):
                        nc.tensor.matmul(
                            q_ps, lhsT=wq_sb[:, dc, pr * P:(pr + 1) * P],
                            rhs=xq_t[:, dc, :],
                            start=(dc == 0), stop=(dc == DC - 1),
                        )
                    nc.vector.tensor_copy(
                        out=qt_sb[:, pr, st * 512:(st + 1) * 512], in_=q_ps
                    )

            # ---- K.T projection ---------------------------------------
            for kt, w in enumerate(kt_widths):
                xk_t = xstream.tile([P, DC, 512], bf16, tag="xq")
                nc.gpsimd.dma_start(
                    xk_t[:, :, :w], xk_r[:, :, kt * 512:kt * 512 + w]
                )
                for pr in range(NPAIR):
                    k_ps = ps_big.tile([P, 512], f32, tag="big")
                    for dc in range(DC):
                        nc.tensor.matmul(
                            k_ps[:, :w], lhsT=wk_sb[:, dc, pr * P:(pr + 1) * P],
                            rhs=xk_t[:, dc, :w],
                            start=(dc == 0), stop=(dc == DC - 1),
                        )
                    nc.vector.tensor_copy(
                        out=kt_sb[:, pr, kt * 512:kt * 512 + w], in_=k_ps[:, :w]
                    )

            # ---- attention per head pair ------------------------------
            for pr in range(NPAIR):
                for qt in range(QT_TILES):
                    qsl = slice(qt * 512, (qt + 1) * 512)
                    av_ps = ps_av.tile([65, 1024], f32, tag="av")
                    for kc in range(nkc):
                        ksl = slice(kc * P, (kc + 1) * P)
                        sc_ps = ps_big.tile([P, 1024], f32, tag="big")
                        # scores.T for head A (rows 0-63) and B (rows 64-127)
                        nc.tensor.matmul(
                            sc_ps[:, 0:512],
                            lhsT=kt_sb[0:64, pr, ksl], rhs=qt_sb[0:64, pr, qsl],
                        )
                        nc.tensor.matmul(
                            sc_ps[:, 512:1024],
                            lhsT=kt_sb[64:128, pr, ksl], rhs=qt_sb[64:128, pr, qsl],
                        )
                        exps = work.tile([P, 1024], bf16, tag="exps")
                        nc.scalar.activation(
                            out=exps, in_=sc_ps, func=EXP,
                            bias=mask_sb[:, kc:kc + 1], scale=1.0,
                        )
                        nc.tensor.matmul(
                            av_ps[0:65, 0:512],
                            lhsT=v_sb[:, kc, pr, 0:65], rhs=exps[:, 0:512],
                            start=(kc == 0), stop=(kc == nkc - 1),
                        )
                        nc.tensor.matmul(
                            av_ps[0:65, 512:1024],
                            lhsT=v_sb[:, kc, pr, 65:130], rhs=exps[:, 512:1024],
                            start=(kc == 0), stop=(kc == nkc - 1),
                        )
                    # normalize: rows 64 hold the softmax denominators.
                    # broadcast denom across 64 partitions via PE ones-matmul,
                    # then reciprocal on the broadcast tile.
                    dn_bf = work.tile([65, 1024], bf16, tag="dnbf")
                    nc.vector.tensor_copy(
                        out=dn_bf[64:65, :], in_=av_ps[64:65, :]
                    )
                    bc_ps = ps_big.tile([P, 1024], f32, tag="big")
                    nc.tensor.matmul(
                        bc_ps[0:64, 0:512],
                        lhsT=ones_sb[64:65, 0:64],
                        rhs=dn_bf[64:65, 0:512],
                    )
                    nc.tensor.matmul(
                        bc_ps[0:64, 512:1024],
                        lhsT=ones_sb[64:65, 0:64],
                        rhs=dn_bf[64:65, 512:1024],
                    )
                    bc_sb = work.tile([64, 1024], f32, tag="bc")
                    nc.vector.reciprocal_approx_fast(out=bc_sb, in_=bc_ps[0:64, :])
                    nc.vector.tensor_mul(
                        out=ot_sb[0:64, pr, qsl],
                        in0=av_ps[0:64, 0:512], in1=bc_sb[:, 0:512],
                    )
                    otB = work.tile([64, 512], bf16, tag="otB")
                    nc.vector.tensor_mul(
                        out=otB,
                        in0=av_ps[0:64, 512:1024], in1=bc_sb[:, 512:1024],
                    )
                    nc.sync.dma_start(out=ot_sb[64:128, pr, qsl], in_=otB)

            # ---- output projection (row-parallel partial) -------------
            for sch in range(S // P):
                y_ps = ps_big.tile([P, D], f32, tag="big")
                for pr in range(NPAIR):
                    for nh in range(2):
                        nc.tensor.matmul(
                            y_ps[:, nh * 512:(nh + 1) * 512],
                            lhsT=ot_sb[:, pr, sch * P:(sch + 1) * P],
                            rhs=wo_sb[:, pr, nh * 512:(nh + 1) * 512],
                            start=(pr == 0), stop=(pr == NPAIR - 1),
                        )
                y_sb = work.tile([P, D], f32, tag="ysb")
                nc.vector.tensor_copy(out=y_sb, in_=y_ps)
                nc.sync.dma_start(out=y_r[:, sch, :], in_=y_sb)

    nc.finalize()
    return nc


def _prep_core_inputs(inputs, nkc):
    """Host-side shard prep: per-core input dict for core c = (batch, group)."""
    vlp = nkc * P
    q = np.asarray(inputs["queries"], np.float32)
    k = np.asarray(inputs["keys"], np.float32)
    v = np.asarray(inputs["values"], np.float32)
    vl = np.asarray(inputs["valid_lens"]).astype(np.int64)
    Wq = np.asarray(inputs["Wq"], np.float32)
    Wk = np.asarray(inputs["Wk"], np.float32)
    Wv = np.asarray(inputs["Wv"], np.float32)
    Wo = np.asarray(inputs["Wo"], np.float32)

    in_maps = []
    for c in range(8):
        b, g = c // 2, c % 2
        rows = slice(g * DG, (g + 1) * DG)
        kpos = np.arange(P)[:, None] + P * np.arange(nkc)[None, :]
        mask = np.where(kpos < vl[b], 0.0, MASK_VALUE).astype(np.float32)
        in_maps.append({
            "xq": np.ascontiguousarray(q[b].T).astype(_BF16),
            "xk": np.ascontiguousarray(k[b, :vlp].T).astype(_BF16),
            "xv": np.ascontiguousarray(v[b, :vlp].T).astype(_BF16),
            "wq": np.ascontiguousarray(Wq[rows].T / 8.0).astype(_BF16),
            "wk": np.ascontiguousarray(Wk[rows].T).astype(_BF16),
            "wv": np.ascontiguousarray(Wv[rows].T).astype(_BF16),
            "wo": np.ascontiguousarray(Wo[:, rows].T).astype(_BF16),
            "maskd": mask,
        })
    return in_maps


def kernel(**inputs):
    from concourse.bass_utils import run_bass_kernel_spmd

    vl = np.asarray(inputs["valid_lens"]).astype(np.int64)
    nkc = int(min(S, max(1, int(vl.max()))) + P - 1) // P

    if nkc not in _build_cache:
        _build_cache[nkc] = _build_nc(nkc)
    nc = _build_cache[nkc]

    in_maps = _prep_core_inputs(inputs, nkc)
    res = run_bass_kernel_spmd(
        nc, in_maps, core_ids=list(range(8)),
        trace=bool(int(os.environ.get("MHA_TRACE", "0"))),
    )
    out = np.empty((B, S, D), np.float32)
    for b in range(B):
        out[b] = res.results[2 * b]["y"] + res.results[2 * b + 1]["y"]
    kernel.last_results = res
    return out


if __name__ == "__main__":
    rng = np.random.default_rng(0)
    ins = {
        "queries": rng.standard_normal((B, S, D), np.float32),
        "keys": rng.standard_normal((B, S, D), np.float32),
        "values": rng.standard_normal((B, S, D), np.float32),
        "valid_lens": np.array([288, 576, 1749, 255], np.int32),
        "Wq": rng.uniform(-1 / 32, 1 / 32, (D, D)).astype(np.float32),
        "Wk": rng.uniform(-1 / 32, 1 / 32, (D, D)).astype(np.float32),
        "Wv": rng.uniform(-1 / 32, 1 / 32, (D, D)).astype(np.float32),
        "Wo": rng.uniform(-1 / 32, 1 / 32, (D, D)).astype(np.float32),
    }
    out = kernel(**ins)
    print("kernel ran, out", out.shape, out.dtype, float(np.abs(out).mean()))
